# revision 10
# baseline (speedup 1.0000x reference)
"""Trainium2 Bass kernel for nn_HLSTransformer (2-block self-attention encoder).

Contract: kernel(**inputs) takes FULL inputs, returns FULL output [32, 1] f32.
Data-parallel over batch: 4 samples per core x 8 cores.

v2 (unmasked fast path):
  - Per-(pair, sample, chunk) attention pipeline.  Scores are symmetric
    (no mask, and the softmax max-shift cancels), so the softmax
    denominator Z[m] equals the ROW sum of e — which the ACT engine
    produces for free via the ACTIVATE accumulator (accum_out) on the
    very exp instruction that computes e.  This removes both the
    ones-column PE matmul streams and the single-partition Ln/Exp
    reciprocal chain of the old kernel.
  - 1/Z via DVE iterative-divide reciprocal, partition->free transpose
    plus row broadcast fused into one identity matmul per 128-chunk.
  - sample0/sample1 matmuls adjacent in the PE queue with disjoint
    row/col groups -> concurrent quadrant execution.
  - Full [128,128] PE transposes (8/pair-block) instead of 16 [64,128].

Masked inputs fall back to the v1 kernel (symmetry does not hold).
All on-device activations in "T layout": [H=64 partitions, N=1024 free],
two samples per 128-partition tile (rows 0-63 / 64-127).
LayerNorm over the whole [N, H] slab per sample -> scalar mean/var;
rsqrt(var+eps) = exp(-0.5*ln(var+eps)) keeps ACT in one table set.
"""

import sys

import numpy as np
import ml_dtypes

if "/opt/trn_rl_repo" not in sys.path:
    sys.path.insert(0, "/opt/trn_rl_repo")

import concourse.bass as bass
import concourse.bacc as bacc
import concourse.tile as tile
from concourse import mybir
from concourse.bass_utils import run_bass_kernel_spmd

F32 = mybir.dt.float32
BF16 = mybir.dt.bfloat16
AF = mybir.ActivationFunctionType

# Force Exp and Ln to resolve to the one table set containing both, so the
# ACT engine never thrashes ACT_TABLE_LOADs between them.
_orig_gat = bacc.get_activation_tables
def _gat_patched(arch):
    out = {}
    for name, fns in _orig_gat(arch).items():
        fns = set(fns)
        if name != "natural_log_exp_and_others":
            fns.discard(mybir.ActivationFunctionType.Exp)
            fns.discard(mybir.ActivationFunctionType.Ln)
        out[name] = fns
    return out
bacc.get_activation_tables = _gat_patched
ALU = mybir.AluOpType

B, N, F_IN, H = 32, 1024, 256, 64
NCORES = 8
S = B // NCORES            # samples per core
NPAIR = S // 2             # sample pairs per core
EPS = 1e-5
SCALE = float(1.0 / np.sqrt(np.float32(N)))
NH = 512                   # free-dim half (PSUM bank)
NC = 8                     # 128-chunks along N


# --------------------------------------------------------------------------
# v2 (fast, unmasked) build
# --------------------------------------------------------------------------

# Chunks whose exp runs on DVE via the Schraudolph bf16 bit trick instead of
# ACT (per (si, c)); balances the ACT-bound exp stream against DVE slack.
FEXP_DVE = {(0, 2), (0, 5), (1, 1), (1, 4), (1, 6)}
# bits16(exp(g*SCALE)) ~= g*FEA + FEB, truncated to int16, bitcast to bf16.
FEA = float(128.0 * np.log2(np.e) * (1.0 / 32.0))
FEB = float(127 * 128 - 5.0)


def _layernorm2(nc, small, psp, selsum, selbc, eps2, v_pair, out_pair, gb):
    """out = (v - mu) * rho per sample (pair tile [128, 1024]); optional
    gamma/beta affine."""
    st6 = small.tile([128, 12], F32, tag="st6")
    nc.vector.bn_stats(st6[:, 0:6], v_pair[:, 0:NH])
    nc.vector.bn_stats(st6[:, 6:12], v_pair[:, NH:N])
    ag = small.tile([128, 2], F32, tag="ag")
    nc.vector.bn_aggr(ag[:, :], st6[:, :])
    m2 = small.tile([128, 2], F32, tag="m2")        # [mean_p, ex2_p]
    nc.vector.tensor_copy(m2[:, 0:1], ag[:, 0:1])
    msqp = small.tile([128, 1], F32, tag="msqp")
    nc.vector.tensor_tensor(msqp[:, :], ag[:, 0:1], ag[:, 0:1], op=ALU.mult)
    nc.vector.tensor_tensor(m2[:, 1:2], ag[:, 1:2], msqp[:, :], op=ALU.add)
    # partition-reduce via selsum (entries 1/64 -> direct average)
    stt = psp.tile([128, NH], F32, tag="ps512")
    nc.tensor.matmul(stt[0:2, 0:2], selsum[:, :], m2[:, :])
    sv = small.tile([2, 2], F32, tag="sv")          # (mean, ex2) per sample
    nc.vector.tensor_copy(sv[:, :], stt[0:2, 0:2])
    pb = small.tile([2, 2], F32, tag="pb")          # cols: [negmu, rho]
    msq = small.tile([2, 1], F32, tag="msq")
    nc.vector.tensor_tensor(msq[:, :], sv[:, 0:1], sv[:, 0:1], op=ALU.mult)
    var = small.tile([2, 1], F32, tag="var")
    nc.vector.tensor_tensor(var[:, :], sv[:, 1:2], msq[:, :], op=ALU.subtract)
    lnv = small.tile([2, 1], F32, tag="lnv")
    nc.scalar.activation(lnv[:, :], var[:, :], AF.Ln, bias=eps2[:, :])
    nc.scalar.activation(pb[:, 1:2], lnv[:, :], AF.Exp, scale=-0.5)
    nc.vector.tensor_scalar(pb[:, 0:1], sv[:, 0:1], -1.0, None, op0=ALU.mult)
    bct = psp.tile([128, NH], F32, tag="ps512")
    nc.tensor.matmul(bct[:, 0:2], selbc[:, :], pb[:, :])
    bc = small.tile([128, 2], F32, tag="bc")
    nc.vector.tensor_copy(bc[:, :], bct[:, 0:2])
    nc.vector.tensor_scalar(
        out_pair[:, :], v_pair[:, :], bc[:, 0:1], bc[:, 1:2],
        op0=ALU.add, op1=ALU.mult,
    )
    if gb is not None:
        gam, bet = gb
        nc.vector.tensor_tensor(out_pair[:, :], out_pair[:, :], gam[:, :], op=ALU.mult)
        nc.vector.tensor_tensor(out_pair[:, :], out_pair[:, :], bet[:, :], op=ALU.add)


def build_nc_fast(use_gb: bool) -> bass.Bass:
    nc = bacc.Bacc("TRN2", target_bir_lowering=False, debug=False, num_devices=NCORES)

    xT = nc.declare_dram_parameter("xT", [S, 2, 128, N], BF16, isOutput=False)
    we = nc.declare_dram_parameter("We", [2, 128, H], BF16, isOutput=False)
    w0 = nc.declare_dram_parameter("W0s", [128, H], BF16, isOutput=False)
    w1 = nc.declare_dram_parameter("W1s", [128, H], BF16, isOutput=False)
    wout = nc.declare_dram_parameter("Wouts", [128, 1], F32, isOutput=False)
    be2 = nc.declare_dram_parameter("be2", [128, 1], F32, isOutput=False)
    b02 = nc.declare_dram_parameter("b02", [128, 1], F32, isOutput=False)
    b12 = nc.declare_dram_parameter("b12", [128, 1], F32, isOutput=False)
    boutp = nc.declare_dram_parameter("bout", [1, 1], F32, isOutput=False)
    ident = nc.declare_dram_parameter("ident", [128, 128], BF16, isOutput=False)
    identf = nc.declare_dram_parameter("identf", [128, 128], F32, isOutput=False)
    selsum_d = nc.declare_dram_parameter("selsum", [128, 2], F32, isOutput=False)
    selbc_d = nc.declare_dram_parameter("selbc", [2, 128], F32, isOutput=False)
    if use_gb:
        gT2_d = nc.declare_dram_parameter("gT2", [128, N], F32, isOutput=False)
        bT2_d = nc.declare_dram_parameter("bT2", [128, N], F32, isOutput=False)
    out_d = nc.declare_dram_parameter("out", [S, 1], F32, isOutput=True)

    with tile.TileContext(nc) as tc:
        with (
            tc.tile_pool(name="consts", bufs=1) as cp,
            tc.tile_pool(name="xt", bufs=4) as xp,
            tc.tile_pool(name="big", bufs=2) as bigp,
            tc.tile_pool(name="hn", bufs=2) as hnp,
            tc.tile_pool(name="e", bufs=3) as ep,
            tc.tile_pool(name="small", bufs=4) as small,
            tc.tile_pool(name="psg", bufs=2, space="PSUM") as pg,
            tc.tile_pool(name="psu", bufs=1, space="PSUM") as pu,
            tc.tile_pool(name="ps512", bufs=2, space="PSUM") as psp,
        ):
            # ---- constants ----
            we_sb = cp.tile([128, 2, H], BF16, tag="we")
            nc.sync.dma_start(we_sb[:, :, :], we.rearrange("k p m -> p k m"))
            w0_sb = cp.tile([128, H], BF16, tag="w0")
            nc.sync.dma_start(w0_sb[:, :], w0[:, :])
            w1_sb = cp.tile([128, H], BF16, tag="w1")
            nc.sync.dma_start(w1_sb[:, :], w1[:, :])
            wo_sb = cp.tile([128, 1], F32, tag="wo")
            nc.sync.dma_start(wo_sb[:, :], wout[:, :])
            be_sb = cp.tile([128, 1], F32, tag="be")
            nc.sync.dma_start(be_sb[:, :], be2[:, :])
            b0_sb = cp.tile([128, 1], F32, tag="b0")
            nc.sync.dma_start(b0_sb[:, :], b02[:, :])
            b1_sb = cp.tile([128, 1], F32, tag="b1")
            nc.sync.dma_start(b1_sb[:, :], b12[:, :])
            bo_sb = cp.tile([1, 1], F32, tag="bo")
            nc.sync.dma_start(bo_sb[:, :], boutp[:, :])
            id_sb = cp.tile([128, 128], BF16, tag="id")
            nc.sync.dma_start(id_sb[:, :], ident[:, :])
            idf_sb = cp.tile([128, 128], F32, tag="idf")
            nc.sync.dma_start(idf_sb[:, :], identf[:, :])
            selsum = cp.tile([128, 2], F32, tag="ss")
            nc.sync.dma_start(selsum[:, :], selsum_d[:, :])
            selbc = cp.tile([2, 128], F32, tag="sb")
            nc.sync.dma_start(selbc[:, :], selbc_d[:, :])
            eps2 = cp.tile([2, 1], F32, tag="eps")
            nc.vector.memset(eps2[:, :], EPS)
            gb = None
            if use_gb:
                gam = cp.tile([128, N], F32, tag="gam")
                nc.sync.dma_start(gam[:, :], gT2_d[:, :])
                bet = cp.tile([128, N], F32, tag="bet")
                nc.sync.dma_start(bet[:, :], bT2_d[:, :])
                gb = (gam, bet)

            # ---- embed: x_embT = relu(We.T @ xT + be), col-tiled pairs ----
            xemb = []
            for p in range(NPAIR):
                emb_ps = pg.tile([128, N], F32, tag="gram")
                for si, s in enumerate((2 * p, 2 * p + 1)):
                    xa = xp.tile([128, N], BF16, tag="xt")
                    xb = xp.tile([128, N], BF16, tag="xt")
                    nc.sync.dma_start(xa[:, :], xT[s, 0, :, :])
                    nc.sync.dma_start(xb[:, :], xT[s, 1, :, :])
                    for nh in range(2):
                        for k, xk in enumerate((xa, xb)):
                            nc.tensor.matmul(
                                emb_ps[64 * si:64 * si + 64, NH * nh:NH * nh + NH],
                                we_sb[:, k, :],
                                xk[:, NH * nh:NH * nh + NH],
                                start=(k == 0), stop=(k == 1),
                                tile_position=(0, 64 * si),
                            )
                xe = bigp.tile([128, N], BF16, tag="xemb", bufs=2)
                nc.vector.tensor_scalar(
                    xe[:, :], emb_ps[:, :], be_sb[:, :], 0.0,
                    op0=ALU.add, op1=ALU.max,
                )
                xemb.append(xe)

            # ---- two transformer blocks (pair-software-pipelined) ----
            def emit_trans(st):
                """hT -> hn (normal layout [128, c, 128]) via full transposes."""
                hT = st["hT"]
                hn = hnp.tile([128, NC, 128], BF16, tag="hn")
                tps = psp.tile([128, NH], F32, tag="ps512")
                tpb = tps.bitcast(BF16).rearrange("p (c m) -> p c m", c=NC)
                for c in range(NC):
                    nc.tensor.transpose(
                        tpb[:, c, :], hT[:, 128 * c:128 * c + 128], id_sb[:, :],
                    )
                nc.vector.tensor_copy(hn[:, :, :], tpb)
                st["hn"] = hn

            def emit_chunks(st):
                """per (sample, chunk): gram -> exp(+Z) -> U (skewed)."""
                hT, hn = st["hT"], st["hn"]
                zt = small.tile([128, 2, NC], F32, tag="zt", bufs=2)
                u_ps = pu.tile([128, N], F32, tag="u")
                elist = {}
                SKEW = 2
                for t in range(NC + SKEW):
                    if t < NC:
                        c = t
                        g0 = pg.tile([128, N], F32, tag="gram")
                        g1 = pg.tile([128, N], F32, tag="gram")
                        gs = (g0, g1)
                        for nh in range(2):
                            ns = slice(NH * nh, NH * nh + NH)
                            for si in range(2):
                                rs = slice(64 * si, 64 * si + 64)
                                nc.tensor.matmul(
                                    gs[si][:, ns],
                                    hT[rs, 128 * c:128 * c + 128],
                                    hT[rs, ns],
                                    start=True, stop=True,
                                )
                        for si in range(2):
                            e = ep.tile([128, N], BF16, tag="e", bufs=12)
                            if (si, c) in FEXP_DVE:
                                nc.vector.tensor_scalar(
                                    e.bitcast(mybir.dt.int16)[:, :],
                                    gs[si][:, :], FEA, FEB,
                                    op0=ALU.mult, op1=ALU.add,
                                )
                                scr = small.tile([128, N], BF16, tag="scr", bufs=2)
                                nc.vector.tensor_scalar(
                                    scr[:, :], e[:, :], 1.0, 0.0,
                                    op0=ALU.mult, op1=ALU.add,
                                    accum_out=zt[:, si, c:c + 1],
                                )
                            else:
                                nc.scalar.activation(
                                    e[:, :], gs[si][:, :], AF.Exp, scale=SCALE,
                                    accum_out=zt[:, si, c:c + 1],
                                )
                            elist[(si, c)] = e
                    if t >= SKEW:
                        c = t - SKEW
                        for nh in range(2):
                            ns = slice(NH * nh, NH * nh + NH)
                            for si in range(2):
                                nc.tensor.matmul(
                                    u_ps[64 * si:64 * si + 64, ns],
                                    hn[:, c, 64 * si:64 * si + 64],
                                    elist[(si, c)][:, ns],
                                    start=(c == 0), stop=(c == NC - 1),
                                    tile_position=(0, 64 * si),
                                    skip_group_check=True,
                                )
                st["zt"], st["u_ps"] = zt, u_ps

            def emit_v(st):
                """normalize columns v = U * (1/Z broadcast); + residual."""
                zt, u_ps, p = st["zt"], st["u_ps"], st["p"]
                rz = small.tile([128, 2, NC], F32, tag="rz", bufs=2)
                nc.vector.reciprocal(rz[:, :, :], zt[:, :, :])
                # lf[:, c, si, i] = rz[:, si, c]  (free-broadcast copy)
                lf = small.tile([128, NC, 2, 64], F32, tag="lf", bufs=2)
                rzb = rz.transpose([0, 2, 1]).unsqueeze(3).broadcast_to(
                    [128, NC, 2, 64])
                nc.vector.tensor_copy(lf[:, :, :, :], rzb)
                v_pair = bigp.tile([128, N], BF16, tag="v")
                for nh in range(2):
                    ns = slice(NH * nh, NH * nh + NH)
                    bps = psp.tile([128, NH], F32, tag="ps512")
                    for j in range(4):
                        c = 4 * nh + j
                        nc.tensor.matmul(
                            bps[:, 128 * j:128 * j + 128],
                            lf[:, c, :, :],
                            idf_sb[:, :],
                            start=True, stop=True,
                        )
                    bsb = small.tile([128, NH], F32, tag="bsb", bufs=2)
                    nc.vector.tensor_copy(bsb[:, :], bps[:, :])
                    nc.vector.tensor_tensor(
                        v_pair[:, ns], u_ps[:, ns], bsb[:, :], op=ALU.mult,
                    )
                nc.gpsimd.tensor_tensor(
                    v_pair[:, :], v_pair[:, :], xemb[p][:, :], op=ALU.add,
                )
                st["v_pair"] = v_pair

            def emit_ln_ffn(st, wf_sb, bf_sb):
                """LN1 -> qkv; FFN; LN2 -> fc."""
                v_pair = st["v_pair"]
                qkv = bigp.tile([128, N], BF16, tag="qkv")
                _layernorm2(nc, small, psp, selsum, selbc, eps2, v_pair, qkv, gb)
                f_pair = bigp.tile([128, N], BF16, tag="f")
                for nh in range(2):
                    ns = slice(NH * nh, NH * nh + NH)
                    fps = psp.tile([128, NH], F32, tag="ps512")
                    nc.tensor.matmul(
                        fps[0:64, :], wf_sb[0:64, :], qkv[0:64, ns],
                        start=True, stop=True,
                    )
                    nc.tensor.matmul(
                        fps[64:128, :], wf_sb[64:128, :], qkv[64:128, ns],
                        start=True, stop=True, tile_position=(64, 64),
                    )
                    nc.vector.tensor_scalar(
                        f_pair[:, ns], fps[:, :], bf_sb[:, :], 0.0,
                        op0=ALU.add, op1=ALU.max,
                    )
                nc.gpsimd.tensor_tensor(
                    f_pair[:, :], f_pair[:, :], qkv[:, :], op=ALU.add,
                )
                fc = bigp.tile([128, N], BF16, tag="fc")
                _layernorm2(nc, small, psp, selsum, selbc, eps2, f_pair, fc, gb)
                st["fc"] = fc

            h_cur = list(xemb)
            for blk in range(2):
                wf_sb = w0_sb if blk == 0 else w1_sb
                bf_sb = b0_sb if blk == 0 else b1_sb
                s0 = {"p": 0, "hT": h_cur[0]}
                s1 = {"p": 1, "hT": h_cur[1]}
                emit_trans(s0)
                emit_chunks(s0)
                emit_trans(s1)      # PE stays busy while DVE starts s0's rz
                emit_v(s0)
                emit_chunks(s1)     # overlaps s0's normalize/LN tail
                emit_ln_ffn(s0, wf_sb, bf_sb)
                emit_v(s1)
                emit_ln_ffn(s1, wf_sb, bf_sb)
                h_cur = [s0["fc"], s1["fc"]]

            # ---- pool + head ----
            for p in range(NPAIR):
                pooled = small.tile([128, 1], F32, tag="pool", bufs=2)
                nc.vector.reduce_sum(
                    pooled[:, :], h_cur[p][:, :], axis=mybir.AxisListType.X,
                )
                sc = psp.tile([128, NH], F32, tag="ps512")
                nc.tensor.matmul(
                    sc[0:1, 0:1], wo_sb[0:64, :], pooled[0:64, :],
                    start=True, stop=True,
                )
                nc.tensor.matmul(
                    sc[0:1, 1:2], wo_sb[64:128, :], pooled[64:128, :],
                    start=True, stop=True,
                )
                # leaky_relu(z + bout) = max(z + bout, 0.01 * (z + bout))
                zb = small.tile([1, 2], F32, tag="zb", bufs=2)
                nc.vector.tensor_scalar(
                    zb[:, :], sc[0:1, 0:2], bo_sb[0:1, :], None, op0=ALU.add,
                )
                res = small.tile([1, 2], F32, tag="res", bufs=2)
                nc.vector.tensor_scalar(
                    res[:, :], zb[:, :], 0.01, None, op0=ALU.mult,
                )
                nc.vector.tensor_tensor(
                    res[:, :], res[:, :], zb[:, :], op=ALU.max,
                )
                for si in range(2):
                    nc.sync.dma_start(
                        out_d[2 * p + si:2 * p + si + 1, :], res[0:1, si:si + 1],
                    )

    nc.compile()
    return nc


# --------------------------------------------------------------------------
# v1 (masked fallback) build — unchanged from the previous kernel
# --------------------------------------------------------------------------

def _ln_scalar_chain(nc, small, pmisc, selsum, selbc, eps2, m2):
    stps = pmisc.tile([2, 2], F32, tag="misc")
    nc.tensor.matmul(stps[:, :], selsum[:, :], m2[:, :])
    sv = small.tile([2, 2], F32, tag="sv")
    nc.vector.tensor_copy(sv[:, :], stps[:, :])
    pb = small.tile([2, 2], F32, tag="pb")
    msq = small.tile([2, 1], F32, tag="msq")
    nc.vector.tensor_tensor(msq[:, :], sv[:, 0:1], sv[:, 0:1], op=ALU.mult)
    var = small.tile([2, 1], F32, tag="var")
    nc.vector.tensor_tensor(var[:, :], sv[:, 1:2], msq[:, :], op=ALU.subtract)
    lnv = small.tile([2, 1], F32, tag="lnv")
    nc.scalar.activation(lnv[:, :], var[:, :], AF.Ln, bias=eps2[:, :])
    nc.scalar.activation(pb[:, 1:2], lnv[:, :], AF.Exp, scale=-0.5)
    nc.vector.tensor_scalar(pb[:, 0:1], sv[:, 0:1], -1.0, None, op0=ALU.mult)
    bcps = pmisc.tile([128, 2], F32, tag="misc")
    nc.tensor.matmul(bcps[:, :], selbc[:, :], pb[:, :])
    bc = small.tile([128, 2], F32, tag="bc")
    nc.vector.tensor_copy(bc[:, :], bcps[:, :])
    return bc


def _layernorm(nc, small, pmisc, selsum, selbc, eps2, v_pair, out_pair, gb):
    st6 = small.tile([128, 12], F32, tag="st6")
    nc.vector.bn_stats(st6[:, 0:6], v_pair[:, 0:NH])
    nc.vector.bn_stats(st6[:, 6:12], v_pair[:, NH:N])
    ag = small.tile([128, 2], F32, tag="ag")
    nc.vector.bn_aggr(ag[:, :], st6[:, :])
    m2 = small.tile([128, 2], F32, tag="m2")
    nc.vector.tensor_copy(m2[:, 0:1], ag[:, 0:1])
    msqp = small.tile([128, 1], F32, tag="msqp")
    nc.vector.tensor_tensor(msqp[:, :], ag[:, 0:1], ag[:, 0:1], op=ALU.mult)
    nc.vector.tensor_tensor(m2[:, 1:2], ag[:, 1:2], msqp[:, :], op=ALU.add)
    bc = _ln_scalar_chain(nc, small, pmisc, selsum, selbc, eps2, m2)
    nc.vector.tensor_scalar(
        out_pair[:, :], v_pair[:, :], bc[:, 0:1], bc[:, 1:2],
        op0=ALU.add, op1=ALU.mult,
    )
    if gb is not None:
        gam, bet = gb
        nc.vector.tensor_tensor(out_pair[:, :], out_pair[:, :], gam[:, :], op=ALU.mult)
        nc.vector.tensor_tensor(out_pair[:, :], out_pair[:, :], bet[:, :], op=ALU.add)


def build_nc_masked(use_mask: bool, use_gb: bool) -> bass.Bass:
    nc = bacc.Bacc("TRN2", target_bir_lowering=False, debug=False, num_devices=NCORES)

    xT = nc.declare_dram_parameter("xT", [S, 2, 128, N], BF16, isOutput=False)
    we = nc.declare_dram_parameter("We", [2, 128, H], BF16, isOutput=False)
    w0 = nc.declare_dram_parameter("W0s", [128, H], BF16, isOutput=False)
    w1 = nc.declare_dram_parameter("W1s", [128, H], BF16, isOutput=False)
    wout = nc.declare_dram_parameter("Wouts", [128, 1], F32, isOutput=False)
    be2 = nc.declare_dram_parameter("be2", [128, 1], F32, isOutput=False)
    b02 = nc.declare_dram_parameter("b02", [128, 1], F32, isOutput=False)
    b12 = nc.declare_dram_parameter("b12", [128, 1], F32, isOutput=False)
    boutp = nc.declare_dram_parameter("bout", [1, 1], F32, isOutput=False)
    ident = nc.declare_dram_parameter("ident", [128, 128], BF16, isOutput=False)
    selsum_d = nc.declare_dram_parameter("selsum", [128, 2], F32, isOutput=False)
    selbc_d = nc.declare_dram_parameter("selbc", [2, 128], F32, isOutput=False)
    if use_gb:
        gT2_d = nc.declare_dram_parameter("gT2", [128, N], F32, isOutput=False)
        bT2_d = nc.declare_dram_parameter("bT2", [128, N], F32, isOutput=False)
    if use_mask:
        maskT_d = nc.declare_dram_parameter("maskT", [S, N, N], F32, isOutput=False)
    out_d = nc.declare_dram_parameter("out", [S, 1], F32, isOutput=True)

    with tile.TileContext(nc) as tc:
        with (
            tc.tile_pool(name="consts", bufs=1) as cp,
            tc.tile_pool(name="xt", bufs=4) as xp,
            tc.tile_pool(name="big", bufs=2) as bigp,
            tc.tile_pool(name="hn", bufs=4) as hnp,
            tc.tile_pool(name="e", bufs=3) as ep,
            tc.tile_pool(name="small", bufs=4) as small,
            tc.tile_pool(name="psg", bufs=2, space="PSUM") as pg,
            tc.tile_pool(name="psu", bufs=1, space="PSUM") as pu,
            tc.tile_pool(name="psmisc", bufs=2, space="PSUM") as pmisc,
        ):
            we_sb = cp.tile([128, 2, H], BF16, tag="we")
            nc.sync.dma_start(we_sb[:, :, :], we.rearrange("k p m -> p k m"))
            w0_sb = cp.tile([128, H], BF16, tag="w0")
            nc.sync.dma_start(w0_sb[:, :], w0[:, :])
            w1_sb = cp.tile([128, H], BF16, tag="w1")
            nc.sync.dma_start(w1_sb[:, :], w1[:, :])
            wo_sb = cp.tile([128, 1], F32, tag="wo")
            nc.sync.dma_start(wo_sb[:, :], wout[:, :])
            be_sb = cp.tile([128, 1], F32, tag="be")
            nc.sync.dma_start(be_sb[:, :], be2[:, :])
            b0_sb = cp.tile([128, 1], F32, tag="b0")
            nc.sync.dma_start(b0_sb[:, :], b02[:, :])
            b1_sb = cp.tile([128, 1], F32, tag="b1")
            nc.sync.dma_start(b1_sb[:, :], b12[:, :])
            bo_sb = cp.tile([1, 1], F32, tag="bo")
            nc.sync.dma_start(bo_sb[:, :], boutp[:, :])
            id_sb = cp.tile([128, 128], BF16, tag="id")
            nc.sync.dma_start(id_sb[:, :], ident[:, :])
            selsum = cp.tile([128, 2], F32, tag="ss")
            nc.sync.dma_start(selsum[:, :], selsum_d[:, :])
            selbc = cp.tile([2, 128], F32, tag="sb")
            nc.sync.dma_start(selbc[:, :], selbc_d[:, :])
            eps2 = cp.tile([2, 1], F32, tag="eps")
            nc.vector.memset(eps2[:, :], EPS)
            onesb = cp.tile([128, 64], F32, tag="onesb")
            nc.vector.memset(onesb[:, :], 1.0)
            zb128 = cp.tile([128, 1], F32, tag="zb128")
            nc.vector.memset(zb128[:, :], 0.0)
            gb = None
            if use_gb:
                gam = cp.tile([128, N], F32, tag="gam")
                nc.sync.dma_start(gam[:, :], gT2_d[:, :])
                bet = cp.tile([128, N], F32, tag="bet")
                nc.sync.dma_start(bet[:, :], bT2_d[:, :])
                gb = (gam, bet)

            xemb = []
            for p in range(NPAIR):
                emb_ps = pg.tile([128, N], F32, tag="gram")
                for si, s in enumerate((2 * p, 2 * p + 1)):
                    xa = xp.tile([128, N], BF16, tag="xt")
                    xb = xp.tile([128, N], BF16, tag="xt")
                    nc.sync.dma_start(xa[:, :], xT[s, 0, :, :])
                    nc.sync.dma_start(xb[:, :], xT[s, 1, :, :])
                    for nh in range(2):
                        for k, xk in enumerate((xa, xb)):
                            nc.tensor.matmul(
                                emb_ps[64 * si:64 * si + 64, NH * nh:NH * nh + NH],
                                we_sb[:, k, :],
                                xk[:, NH * nh:NH * nh + NH],
                                start=(k == 0), stop=(k == 1),
                                tile_position=(0, 64 * si),
                            )
                xe = bigp.tile([128, N], BF16, tag="xemb", bufs=2)
                nc.vector.tensor_scalar(
                    xe[:, :], emb_ps[:, :], be_sb[:, :], 0.0,
                    op0=ALU.add, op1=ALU.max,
                )
                xemb.append(xe)

            h_cur = list(xemb)
            fc_out = [None] * NPAIR
            for blk in range(2):
                wf_sb = w0_sb if blk == 0 else w1_sb
                bf_sb = b0_sb if blk == 0 else b1_sb
                for p in range(NPAIR):
                    hT = h_cur[p]
                    hn = []
                    for si in range(2):
                        t = hnp.tile([128, NC, 65], BF16, tag="hn")
                        nc.gpsimd.memset(t[:, :, 64:65], 1.0)
                        for cq in range(2):
                            tp = pmisc.tile([128, 256], BF16, tag="misc")
                            for j in range(4):
                                c = 4 * cq + j
                                nc.tensor.transpose(
                                    tp[:, 64 * j:64 * j + 64],
                                    hT[64 * si:64 * si + 64, 128 * c:128 * c + 128],
                                    id_sb[64 * si:64 * si + 64, 64 * si:64 * si + 64],
                                )
                            nc.vector.tensor_copy(
                                t[:, 4 * cq:4 * cq + 4, 0:64],
                                tp.rearrange("p (c m) -> p c m", c=4),
                            )
                        hn.append(t)

                    v_pair = bigp.tile([128, N], BF16, tag="v")
                    for nh in range(2):
                        ns = slice(NH * nh, NH * nh + NH)
                        elist = []
                        for c in range(8):
                            g = pg.tile([128, N], F32, tag="gram")
                            for si in range(2):
                                nc.tensor.matmul(
                                    g[:, NH * si:NH * si + NH],
                                    hT[64 * si:64 * si + 64, 128 * c:128 * c + 128],
                                    hT[64 * si:64 * si + 64, ns],
                                    start=True, stop=True,
                                )
                            if use_mask:
                                for si, s in enumerate((2 * p, 2 * p + 1)):
                                    mt = ep.tile([128, NH], BF16, tag="mt", bufs=3)
                                    nc.sync.dma_start(
                                        mt[:, :],
                                        maskT_d[s, 128 * c:128 * c + 128, ns],
                                    )
                                    nc.vector.tensor_tensor(
                                        g[:, NH * si:NH * si + NH],
                                        g[:, NH * si:NH * si + NH],
                                        mt[:, :], op=ALU.add,
                                    )
                            e = ep.tile([128, N], BF16, tag="e", bufs=12)
                            nc.scalar.activation(e[:, :], g[:, :], AF.Exp, scale=SCALE)
                            elist.append(e)
                        uA = pu.tile([128, NH], F32, tag="uA")
                        uB = pu.tile([128, NH], F32, tag="uB")
                        for c in range(8):
                            e = elist[c]
                            nc.tensor.matmul(
                                uA[0:65, :], hn[0][:, c, 0:65], e[:, 0:NH],
                                start=(c == 0), stop=(c == 7),
                                skip_group_check=True,
                            )
                            nc.tensor.matmul(
                                uB[64:128, :], hn[1][:, c, 0:64], e[:, NH:N],
                                start=(c == 0), stop=(c == 7),
                                tile_position=(0, 64),
                            )
                            nc.tensor.matmul(
                                uA[96:97, :], hn[1][:, c, 64:65], e[:, NH:N],
                                start=(c == 0), stop=(c == 7),
                                tile_position=(0, 96),
                                skip_group_check=True,
                            )
                        zl = small.tile([128, NH], F32, tag="zl", bufs=2)
                        rzs = small.tile([128, NH], F32, tag="rz2", bufs=2)
                        nc.scalar.activation(zl[64:65, :], uA[64:65, :], AF.Ln,
                                             bias=zb128[64:65, :])
                        nc.scalar.activation(zl[96:97, :], uA[96:97, :], AF.Ln,
                                             bias=zb128[96:97, :])
                        nc.scalar.activation(rzs[64:65, :], zl[64:65, :], AF.Exp,
                                             scale=-1.0, bias=zb128[64:65, :])
                        nc.scalar.activation(rzs[96:97, :], zl[96:97, :], AF.Exp,
                                             scale=-1.0, bias=zb128[96:97, :])
                        rzp = pmisc.tile([128, NH], F32, tag="misc")
                        nc.tensor.matmul(
                            rzp[0:64, :], onesb[64:65, :], rzs[64:65, :],
                            start=True, stop=True, tile_position=(64, 0),
                            skip_group_check=True,
                        )
                        nc.tensor.matmul(
                            rzp[64:128, :], onesb[96:97, :], rzs[96:97, :],
                            start=True, stop=True, tile_position=(96, 64),
                            skip_group_check=True,
                        )
                        rzb = small.tile([128, NH], F32, tag="rzb", bufs=2)
                        nc.vector.tensor_copy(rzb[:, :], rzp[:, :])
                        nc.vector.tensor_tensor(
                            v_pair[0:64, ns], uA[0:64, :], rzb[0:64, :], op=ALU.mult,
                        )
                        nc.vector.tensor_tensor(
                            v_pair[64:128, ns], uB[64:128, :], rzb[64:128, :],
                            op=ALU.mult,
                        )
                    nc.gpsimd.tensor_tensor(
                        v_pair[:, :], v_pair[:, :], xemb[p][:, :], op=ALU.add,
                    )
                    qkv = bigp.tile([128, N], BF16, tag="qkv")
                    _layernorm(nc, small, pmisc, selsum, selbc, eps2, v_pair, qkv, gb)

                    f_pair = bigp.tile([128, N], BF16, tag="f")
                    for nh in range(2):
                        ns = slice(NH * nh, NH * nh + NH)
                        fps = pmisc.tile([128, NH], F32, tag="misc")
                        nc.tensor.matmul(
                            fps[0:64, :], wf_sb[0:64, :], qkv[0:64, ns],
                            start=True, stop=True,
                        )
                        nc.tensor.matmul(
                            fps[64:128, :], wf_sb[64:128, :], qkv[64:128, ns],
                            start=True, stop=True, tile_position=(64, 64),
                        )
                        nc.vector.tensor_scalar(
                            f_pair[:, ns], fps[:, :], bf_sb[:, :], 0.0,
                            op0=ALU.add, op1=ALU.max,
                        )
                    nc.gpsimd.tensor_tensor(
                        f_pair[:, :], f_pair[:, :], qkv[:, :], op=ALU.add,
                    )
                    fc = bigp.tile([128, N], BF16, tag="fc")
                    _layernorm(nc, small, pmisc, selsum, selbc, eps2, f_pair, fc, gb)
                    fc_out[p] = fc
                h_cur = list(fc_out)

            for p in range(NPAIR):
                pooled = small.tile([128, 1], F32, tag="pool", bufs=2)
                nc.vector.reduce_sum(
                    pooled[:, :], h_cur[p][:, :], axis=mybir.AxisListType.X,
                )
                sc = pmisc.tile([1, 2], F32, tag="misc")
                nc.tensor.matmul(
                    sc[0:1, 0:1], wo_sb[0:64, :], pooled[0:64, :],
                    start=True, stop=True,
                )
                nc.tensor.matmul(
                    sc[0:1, 1:2], wo_sb[64:128, :], pooled[64:128, :],
                    start=True, stop=True,
                )
                zb = small.tile([1, 2], F32, tag="zb", bufs=2)
                nc.vector.tensor_scalar(
                    zb[:, :], sc[:, :], bo_sb[0:1, :], None, op0=ALU.add,
                )
                res = small.tile([1, 2], F32, tag="res", bufs=2)
                nc.vector.tensor_scalar(
                    res[:, :], zb[:, :], 0.01, None, op0=ALU.mult,
                )
                nc.vector.tensor_tensor(
                    res[:, :], res[:, :], zb[:, :], op=ALU.max,
                )
                for si in range(2):
                    nc.sync.dma_start(
                        out_d[2 * p + si:2 * p + si + 1, :], res[0:1, si:si + 1],
                    )

    nc.compile()
    return nc


_NC_CACHE: dict = {}


def _get_nc(use_mask: bool, use_gb: bool) -> bass.Bass:
    key = (use_mask, use_gb)
    if key not in _NC_CACHE:
        if use_mask:
            _NC_CACHE[key] = build_nc_masked(use_mask, use_gb)
        else:
            _NC_CACHE[key] = build_nc_fast(use_gb)
    return _NC_CACHE[key]


def make_inputs(x, mask, We, be, gamma, beta, W0, b0, W1, b1, Wout, bout,
                use_mask, use_gb):
    ident = np.eye(128, dtype=np.float32)
    selsum = np.zeros((128, 2), dtype=np.float32)
    selsum[0:64, 0] = 1.0 / 64.0
    selsum[64:128, 1] = 1.0 / 64.0
    selbc = np.zeros((2, 128), dtype=np.float32)
    selbc[0, 0:64] = 1.0
    selbc[1, 64:128] = 1.0

    def stack2(v):
        v = np.asarray(v, dtype=np.float32).reshape(-1)
        return np.concatenate([v, v]).reshape(128, 1)

    common = {
        "We": np.ascontiguousarray(np.asarray(We, dtype=np.float32)).reshape(2, 128, H).astype(ml_dtypes.bfloat16),
        "W0s": np.concatenate([W0, W0]).astype(ml_dtypes.bfloat16),
        "W1s": np.concatenate([W1, W1]).astype(ml_dtypes.bfloat16),
        "Wouts": np.concatenate([Wout, Wout]).astype(np.float32),
        "be2": stack2(be), "b02": stack2(b0), "b12": stack2(b1),
        "bout": np.asarray(bout, dtype=np.float32).reshape(1, 1),
        "ident": ident.astype(ml_dtypes.bfloat16), "selsum": selsum, "selbc": selbc,
    }
    if not use_mask:
        common["identf"] = ident
    if use_gb:
        gT = np.ascontiguousarray(np.asarray(gamma, dtype=np.float32).T)
        bT = np.ascontiguousarray(np.asarray(beta, dtype=np.float32).T)
        common["gT2"] = np.concatenate([gT, gT]).astype(np.float32)
        common["bT2"] = np.concatenate([bT, bT]).astype(np.float32)

    in_maps = []
    for k in range(NCORES):
        xs = x[S * k:S * k + S]                       # [S, N, F_IN]
        xTs = np.ascontiguousarray(
            xs.transpose(0, 2, 1)).reshape(S, 2, 128, N)
        m = dict(common)
        m["xT"] = xTs.astype(ml_dtypes.bfloat16)
        if use_mask:
            m["maskT"] = np.ascontiguousarray(
                mask[S * k:S * k + S].transpose(0, 2, 1))
        in_maps.append(m)
    return in_maps


def kernel(x, mask, We, be, gamma, beta, W0, b0, W1, b1, Wout, bout):
    x = np.ascontiguousarray(np.asarray(x, dtype=np.float32))
    mask = np.asarray(mask, dtype=np.float32)
    use_mask = bool(np.any(mask))
    use_gb = bool(np.any(np.asarray(gamma) != 1.0) or np.any(np.asarray(beta)))

    nc = _get_nc(use_mask, use_gb)
    in_maps = make_inputs(x, mask, We, be, gamma, beta, W0, b0, W1, b1,
                          Wout, bout, use_mask, use_gb)

    res = run_bass_kernel_spmd(nc, in_maps, list(range(NCORES)))
    global LAST_RESULT
    LAST_RESULT = res
    out = np.concatenate([res.results[k]["out"] for k in range(NCORES)], axis=0)
    return out.astype(np.float32)


LAST_RESULT = None
